# revision 13
# baseline (speedup 1.0000x reference)
"""Causal self-attention kernel for Trainium2, 8 NeuronCores.

Sharding: DP4 x TP2. Core c = 2*b + g handles batch b (2048 tokens) and
head-group g (8 of 16 heads). Each core:
  - transposes its x slice on the PE (d_model onto partitions),
  - computes Q,K dim-major ([head_dim, tokens]) and V token-major via fp32r
    matmuls against its w_qkv column shard,
  - per head: scores^T = K_h tile^T @ Q_h in [k, q] layout, exp on ACT
    (folding the 1/sqrt(64) scale), causal masking via precomputed 0/1
    mask tiles on the diagonal band (off-diagonal tiles skipped entirely),
  - attention output via probs^T matmuls with V augmented by a ones
    column, which yields the softmax denominator for free; normalization
    through vector reciprocal + gpsimd partition_broadcast,
  - projects with its w_proj row shard (token-major output),
  - pairwise AllReduce (cores 2b, 2b+1) of the projection partials.

Everything (shapes, sharding) is hardcoded for
x: [4, 2048, 1024], w_qkv: [1024, 3072], w_proj: [1024, 1024], f32.
"""

import ml_dtypes
import numpy as np

import concourse.bacc as bacc
import concourse.mybir as mybir
import concourse.tile as tile
from concourse.bass_utils import run_bass_kernel_spmd

F32 = mybir.dt.float32
F32R = mybir.dt.float32r
BF16 = mybir.dt.bfloat16

S = 2048  # tokens per core (one batch element)
D = 1024  # d_model
HL = 8  # heads per core (local)
HD = 64  # head dim
GD = HL * HD  # 512, head-group dim
NQT = S // 512  # 4 q-tiles of 512
NKC = S // 128  # 16 k-chunks of 128
NDM = D // 128  # 8 d_model chunks
NTOK = S // 128  # 16 token tiles of 128

_NC_CACHE = {}


def _phase_qkv(nc, wq_p, xst_p, xtn_p, tps_p, mm_ps, wqkv, xb, identity, qkT, v_sb):
    """Transpose x (PE) and run the QKV matmuls, one 512-token tile at a time."""
    w_sb = wq_p.tile([128, NDM, 3 * GD], F32R)
    for k in range(NDM):
        nc.sync.dma_start(
            out=w_sb[:, k, :],
            in_=wqkv[k * 128 : (k + 1) * 128, :].bitcast(F32R),
        )
    for n in range(NQT):
        xTn = xtn_p.tile([128, NDM, 512], F32R, tag="xTn", name="xTn")
        for t4 in range(4):
            t = n * 4 + t4
            xst = xst_p.tile([128, D], F32R, tag="xst", name="xst")
            nc.sync.dma_start(
                out=xst, in_=xb[t * 128 : (t + 1) * 128, :].bitcast(F32R)
            )
            for half in range(2):
                tps = tps_p.tile([128, 4, 128], F32R, tag="tps", name="tps")
                for q4 in range(4):
                    k = half * 4 + q4
                    nc.tensor.transpose(
                        tps[:, q4], xst[:, k * 128 : (k + 1) * 128], identity
                    )
                nc.vector.tensor_copy(
                    out=xTn[
                        :, half * 4 : (half + 1) * 4, t4 * 128 : (t4 + 1) * 128
                    ],
                    in_=tps,
                )
        # Q and K column blocks (dim-major output)
        for m in range(2 * GD // 128):
            ps = mm_ps.tile([128, 512], F32, tag="mmps", name="qkps")
            for k in range(NDM):
                nc.tensor.matmul(
                    ps,
                    w_sb[:, k, m * 128 : (m + 1) * 128],
                    xTn[:, k, :],
                    start=(k == 0),
                    stop=(k == NDM - 1),
                )
            nc.vector.tensor_copy(out=qkT[:, m, n * 512 : (n + 1) * 512], in_=ps)
        # V block (token-major output), scattered per head
        for t4 in range(4):
            t = n * 4 + t4
            ps = mm_ps.tile([128, 512], F32, tag="mmps", name="vps")
            for k in range(NDM):
                nc.tensor.matmul(
                    ps,
                    xTn[:, k, t4 * 128 : (t4 + 1) * 128],
                    w_sb[:, k, 2 * GD : 3 * GD],
                    start=(k == 0),
                    stop=(k == NDM - 1),
                )
            nc.vector.tensor_copy(
                out=v_sb[:, t, :, 0:HD],
                in_=ps.rearrange("p (h d) -> p h d", h=HL),
            )


def _attn_qtile(nc, probs_p, den_p, attn_ps, y_ps, mask_sb, qkT, v_sb, yT, j):
    """Attention for q-tile j, one head pair (partitions 0-63 / 64-127) at a
    time: the two QK matmuls land in different PE row groups and write two
    banks of one PSUM tile, one wide exp covers both, then two bf16 AVs."""
    for hp in range(HL // 2):
        yps = {}
        for hi in range(2):
            yps[hi] = y_ps.tile(
                [HD + 1, 512], F32, tag=f"yps{hi}", name=f"yps{hi}", bufs=1
            )
        for c in range(4 * j + 4):
            d = c - 4 * j  # >= 0 on the diagonal band
            sps2 = attn_ps.tile([128, 2, 512], F32, tag="sps2", name="sps2")
            for hi in range(2):
                h = 2 * hp + hi
                po = (h % 2) * 64
                nc.tensor.matmul(
                    sps2[:, hi, :],
                    qkT[po : po + 64, 4 + h // 2, c * 128 : (c + 1) * 128],
                    qkT[po : po + 64, h // 2, j * 512 : (j + 1) * 512],
                    start=True,
                    stop=True,
                )
            probs2 = probs_p.tile([128, 2, 512], BF16, tag="probs", name="probs")
            nc.scalar.activation(
                out=probs2,
                in_=sps2,
                func=mybir.ActivationFunctionType.Exp,
                scale=0.125,
            )
            if d >= 0:
                nc.vector.tensor_mul(probs2, probs2, mask_sb[:, d, :, :])
            for hi in range(2):
                h = 2 * hp + hi
                nc.tensor.matmul(
                    yps[hi],
                    v_sb[:, c, h, :],
                    probs2[:, hi, :],
                    start=(c == 0),
                    stop=(c == 4 * j + 3),
                )
        # softmax denominators: copy the ones-row out of PSUM, fast
        # reciprocal, broadcast across partitions, scale y
        for hi in range(2):
            h = 2 * hp + hi
            po = (h % 2) * 64
            den = den_p.tile([1, 512], F32, tag="den", name="den")
            nc.scalar.activation(
                out=den,
                in_=yps[hi][HD : HD + 1, :],
                func=mybir.ActivationFunctionType.Copy,
            )
            nc.vector.reciprocal_approx_fast(out=den, in_=den)
            denb = den_p.tile([HD, 512], F32, tag="denb", name="denb")
            nc.gpsimd.partition_broadcast(denb, den)
            nc.vector.tensor_mul(
                yT[po : po + 64, h // 2, j * 512 : (j + 1) * 512],
                yps[hi][0:HD, :],
                denb,
            )


def _proj_chunk(nc, out_p, mm_ps, wp_sb, yT, cc_in, j):
    """Projection for the 4 token tiles of q-tile j (token-major output)."""
    for mt in range(4 * j, 4 * j + 4):
        osb = out_p.tile([128, D], F32, tag="osb", name="osb")
        for nh in range(2):
            ps = mm_ps.tile([128, 512], F32, tag="mmps", name="ops")
            for kk in range(GD // 128):
                nc.tensor.matmul(
                    ps,
                    yT[:, kk, mt * 128 : (mt + 1) * 128],
                    wp_sb[:, kk, nh * 512 : (nh + 1) * 512],
                    start=(kk == 0),
                    stop=(kk == GD // 128 - 1),
                )
            nc.vector.tensor_copy(out=osb[:, nh * 512 : (nh + 1) * 512], in_=ps)
        nc.sync.dma_start(out=cc_in[mt * 128 : (mt + 1) * 128, :], in_=osb)


def _ar_chunk(nc, cc_in, cc_out, out, j):
    """AllReduce + final output DMA for q-tile j's 512 token rows."""
    rows = slice(j * 512, (j + 1) * 512)
    nc.gpsimd.collective_compute(
        "AllReduce",
        mybir.AluOpType.add,
        replica_groups=[[0, 1], [2, 3], [4, 5], [6, 7]],
        ins=[cc_in[rows, :].opt()],
        outs=[cc_out[rows, :].opt()],
    )
    for mt in range(4 * j, 4 * j + 4):
        nc.sync.dma_start(
            out=out[mt * 128 : (mt + 1) * 128, :],
            in_=cc_out[mt * 128 : (mt + 1) * 128, :],
        )


def _build_nc():
    nc = bacc.Bacc(None, num_devices=8)

    xb = nc.dram_tensor("xb", [S, D], F32, kind="ExternalInput").ap()
    wqkv = nc.dram_tensor("wqkv", [D, 3 * GD], F32, kind="ExternalInput").ap()
    wproj = nc.dram_tensor("wproj", [GD, D], F32, kind="ExternalInput").ap()
    masks = nc.dram_tensor("masks", [128, 4, 1024], BF16, kind="ExternalInput").ap()
    ident = nc.dram_tensor("ident", [128, 128], F32, kind="ExternalInput").ap()
    out = nc.dram_tensor("out", [S, D], F32, kind="ExternalOutput").ap()

    with tile.TileContext(nc) as tc:
        with (
            tc.tile_pool(name="const", bufs=1) as const,
            tc.tile_pool(name="qk_p", bufs=1) as qk_p,
            tc.tile_pool(name="v_p", bufs=1) as v_p,
            tc.tile_pool(name="mm_ps", bufs=2, space="PSUM") as mm_ps,
            tc.tile_pool(name="dram", bufs=1, space="DRAM") as dram,
        ):
            identity = const.tile([128, 128], F32R)
            nc.sync.dma_start(out=identity, in_=ident.bitcast(F32R))
            mask_sb = const.tile([128, 4, 2, 512], BF16)
            nc.sync.dma_start(
                out=mask_sb, in_=masks.rearrange("p d (two q) -> p d two q", two=2)
            )

            # Q rows (m 0..3) and K rows (m 4..7), dim-major [512, 2048] each
            qkT = qk_p.tile([128, 2 * GD // 128, S], BF16)
            # V token-major with a ones column per head: [tok_tile, head, 65]
            v_sb = v_p.tile([128, NTOK, HL, HD + 1], BF16)
            nc.vector.memset(v_sb[:, :, :, HD : HD + 1], 1.0)

            with (
                tc.tile_pool(name="wq_p", bufs=1) as wq_p,
                tc.tile_pool(name="xst_p", bufs=3) as xst_p,
                tc.tile_pool(name="xtn_p", bufs=2) as xtn_p,
                tc.tile_pool(name="tps_p", bufs=2, space="PSUM") as tps_p,
            ):
                _phase_qkv(
                    nc, wq_p, xst_p, xtn_p, tps_p, mm_ps,
                    wqkv, xb, identity, qkT, v_sb,
                )

            with (
                tc.tile_pool(name="yt_p", bufs=1) as yt_p,
                tc.tile_pool(name="probs_p", bufs=6) as probs_p,
                tc.tile_pool(name="den_p", bufs=4) as den_p,
                tc.tile_pool(name="wp_p", bufs=1) as wp_p,
                tc.tile_pool(name="out_p", bufs=3) as out_p,
                tc.tile_pool(name="y_ps", bufs=1, space="PSUM") as y_ps,
                tc.tile_pool(name="attn_ps", bufs=2, space="PSUM") as attn_ps,
            ):
                yT = yt_p.tile([128, GD // 128, S], F32R)
                wp_sb = wp_p.tile([128, GD // 128, D], F32R)
                for kk in range(GD // 128):
                    nc.sync.dma_start(
                        out=wp_sb[:, kk, :],
                        in_=wproj[kk * 128 : (kk + 1) * 128, :].bitcast(F32R),
                    )
                cc_in = dram.tile([S, D], F32)
                cc_out = dram.tile([S, D], F32)
                for j in range(NQT):
                    _attn_qtile(
                        nc, probs_p, den_p, attn_ps, y_ps,
                        mask_sb, qkT, v_sb, yT, j,
                    )
                    _proj_chunk(nc, out_p, mm_ps, wp_sb, yT, cc_in, j)
                    _ar_chunk(nc, cc_in, cc_out, out, j)

    nc.compile()
    return nc


def _host_consts():
    ki = np.arange(128)[:, None, None]
    dd = np.arange(4)[None, :, None] * 128
    qj = np.arange(512)[None, None, :]
    m = (qj >= ki + dd).astype(np.float32)  # [128, 4, 512]
    masks = np.concatenate([m, m], axis=-1).astype(ml_dtypes.bfloat16)
    ident = np.eye(128, dtype=np.float32)
    return masks, ident


def _in_maps(x, w_qkv, w_proj):
    masks, ident = _host_consts()
    maps = []
    for c in range(8):
        b, g = c // 2, c % 2
        wq = w_qkv[:, g * GD : (g + 1) * GD]
        wk = w_qkv[:, D + g * GD : D + (g + 1) * GD]
        wv = w_qkv[:, 2 * D + g * GD : 2 * D + (g + 1) * GD]
        maps.append(
            {
                "xb": np.ascontiguousarray(x[b]),
                "wqkv": np.ascontiguousarray(np.concatenate([wq, wk, wv], axis=1)),
                "wproj": np.ascontiguousarray(w_proj[g * GD : (g + 1) * GD, :]),
                "masks": masks,
                "ident": ident,
            }
        )
    return maps


def kernel(x, w_qkv, w_proj):
    x = np.ascontiguousarray(x, dtype=np.float32)
    w_qkv = np.ascontiguousarray(w_qkv, dtype=np.float32)
    w_proj = np.ascontiguousarray(w_proj, dtype=np.float32)
    if "nc" not in _NC_CACHE:
        _NC_CACHE["nc"] = _build_nc()
    nc = _NC_CACHE["nc"]
    r = run_bass_kernel_spmd(nc, _in_maps(x, w_qkv, w_proj), list(range(8)))
    return np.stack([r.results[2 * b]["out"] for b in range(4)], axis=0)


# revision 14
# speedup vs baseline: 1.0438x; 1.0438x over previous
"""Causal self-attention kernel for Trainium2, 8 NeuronCores.

Sharding: DP4 x TP2. Core c = 2*b + g handles batch b (2048 tokens) and
head-group g (8 of 16 heads). Each core:
  - transposes its x slice on the PE (d_model onto partitions),
  - computes Q,K dim-major ([head_dim, tokens]) and V token-major via fp32r
    matmuls against its w_qkv column shard,
  - per head: scores^T = K_h tile^T @ Q_h in [k, q] layout, exp on ACT
    (folding the 1/sqrt(64) scale), causal masking via precomputed 0/1
    mask tiles on the diagonal band (off-diagonal tiles skipped entirely),
  - attention output via probs^T matmuls with V augmented by a ones
    column, which yields the softmax denominator for free; normalization
    through vector reciprocal + gpsimd partition_broadcast,
  - projects with its w_proj row shard (token-major output),
  - pairwise AllReduce (cores 2b, 2b+1) of the projection partials.

Everything (shapes, sharding) is hardcoded for
x: [4, 2048, 1024], w_qkv: [1024, 3072], w_proj: [1024, 1024], f32.
"""

import ml_dtypes
import numpy as np

import concourse.bacc as bacc
import concourse.mybir as mybir
import concourse.tile as tile
from concourse.bass_utils import run_bass_kernel_spmd

F32 = mybir.dt.float32
F32R = mybir.dt.float32r
BF16 = mybir.dt.bfloat16

S = 2048  # tokens per core (one batch element)
D = 1024  # d_model
HL = 8  # heads per core (local)
HD = 64  # head dim
GD = HL * HD  # 512, head-group dim
NQT = S // 512  # 4 q-tiles of 512
NKC = S // 128  # 16 k-chunks of 128
NDM = D // 128  # 8 d_model chunks
NTOK = S // 128  # 16 token tiles of 128

_NC_CACHE = {}


def _phase_qkv(nc, wq_p, xst_p, xtn_p, tps_p, mm_ps, wqkv, xb, identity, qkT, v_sb):
    """Transpose x (PE) and run the QKV matmuls, one 512-token tile at a time."""
    w_sb = wq_p.tile([128, NDM, 3 * GD], F32R)
    for k in range(NDM):
        nc.sync.dma_start(
            out=w_sb[:, k, :],
            in_=wqkv[k * 128 : (k + 1) * 128, :].bitcast(F32R),
        )
    for n in range(NQT):
        xTn = xtn_p.tile([128, NDM, 512], F32R, tag="xTn", name="xTn")
        for t4 in range(4):
            t = n * 4 + t4
            xst = xst_p.tile([128, D], F32R, tag="xst", name="xst")
            nc.sync.dma_start(
                out=xst, in_=xb[t * 128 : (t + 1) * 128, :].bitcast(F32R)
            )
            for half in range(2):
                tps = tps_p.tile([128, 4, 128], F32R, tag="tps", name="tps")
                for q4 in range(4):
                    k = half * 4 + q4
                    nc.tensor.transpose(
                        tps[:, q4], xst[:, k * 128 : (k + 1) * 128], identity
                    )
                nc.vector.tensor_copy(
                    out=xTn[
                        :, half * 4 : (half + 1) * 4, t4 * 128 : (t4 + 1) * 128
                    ],
                    in_=tps,
                )
        # Q and K column blocks (dim-major output)
        for m in range(2 * GD // 128):
            ps = mm_ps.tile([128, 512], F32, tag="mmps", name="qkps")
            for k in range(NDM):
                nc.tensor.matmul(
                    ps,
                    w_sb[:, k, m * 128 : (m + 1) * 128],
                    xTn[:, k, :],
                    start=(k == 0),
                    stop=(k == NDM - 1),
                )
            nc.vector.tensor_copy(out=qkT[:, m, n * 512 : (n + 1) * 512], in_=ps)
        # V block (token-major output), scattered per head
        for t4 in range(4):
            t = n * 4 + t4
            ps = mm_ps.tile([128, 512], F32, tag="mmps", name="vps")
            for k in range(NDM):
                nc.tensor.matmul(
                    ps,
                    xTn[:, k, t4 * 128 : (t4 + 1) * 128],
                    w_sb[:, k, 2 * GD : 3 * GD],
                    start=(k == 0),
                    stop=(k == NDM - 1),
                )
            nc.vector.tensor_copy(
                out=v_sb[:, t, :, 0:HD],
                in_=ps.rearrange("p (h d) -> p h d", h=HL),
            )


def _attn_qtile(nc, probs_p, den_p, attn_ps, y_ps, mask_sb, qkT, v_sb, yT, j):
    """Attention for q-tile j, one head pair (partitions 0-63 / 64-127) at a
    time: the two QK matmuls land in different PE row groups and write two
    banks of one PSUM tile, one wide exp covers both, then two bf16 AVs."""
    for hp in range(HL // 2):
        yps = {}
        for hi in range(2):
            yps[hi] = y_ps.tile(
                [128, 512], F32, tag=f"yps{hi}", name=f"yps{hi}", bufs=1
            )
        for c in range(4 * j + 4):
            d = c - 4 * j  # >= 0 on the diagonal band
            # live q-column range: below off the tile is fully masked
            off = max(d, 0) * 128
            sps2 = attn_ps.tile([128, 2, 512], F32, tag="sps2", name="sps2")
            for hi in range(2):
                h = 2 * hp + hi
                po = (h % 2) * 64
                nc.tensor.matmul(
                    sps2[:, hi, off:512],
                    qkT[po : po + 64, 4 + h // 2, c * 128 : (c + 1) * 128],
                    qkT[po : po + 64, h // 2, j * 512 + off : (j + 1) * 512],
                    start=True,
                    stop=True,
                )
            probs2 = probs_p.tile([128, 2, 512], BF16, tag="probs", name="probs")
            if off:
                nc.vector.memset(probs2[:, :, 0:off], 0.0)
            nc.scalar.activation(
                out=probs2[:, :, off:512],
                in_=sps2[:, :, off:512],
                func=mybir.ActivationFunctionType.Exp,
                scale=0.125,
            )
            if d >= 0:
                nc.vector.tensor_mul(
                    probs2[:, :, off : off + 128],
                    probs2[:, :, off : off + 128],
                    mask_sb[:, d, :, off : off + 128],
                )
            for hi in range(2):
                h = 2 * hp + hi
                nc.tensor.matmul(
                    yps[hi],
                    v_sb[:, c, h, :],
                    probs2[:, hi, :],
                    start=(c == 0),
                    stop=(c == 4 * j + 3),
                )
        # softmax denominators: copy the ones-row out of PSUM, fast
        # reciprocal, broadcast across partitions, scale y
        for hi in range(2):
            h = 2 * hp + hi
            po = (h % 2) * 64
            den = den_p.tile([1, 512], F32, tag="den", name="den")
            nc.scalar.activation(
                out=den,
                in_=yps[hi][HD : HD + 1, :],
                func=mybir.ActivationFunctionType.Copy,
            )
            nc.vector.reciprocal_approx_fast(out=den, in_=den)
            denb = den_p.tile([HD, 512], F32, tag="denb", name="denb")
            nc.gpsimd.partition_broadcast(denb, den)
            nc.vector.tensor_mul(
                yT[po : po + 64, h // 2, j * 512 : (j + 1) * 512],
                yps[hi][0:HD, :],
                denb,
            )


def _proj_chunk(nc, out_p, mm_ps, wp_sb, yT, cc_in, j):
    """Projection for the 4 token tiles of q-tile j (token-major output)."""
    for mt in range(4 * j, 4 * j + 4):
        osb = out_p.tile([128, D], F32, tag="osb", name="osb")
        for nh in range(2):
            ps = mm_ps.tile([128, 512], F32, tag="mmps", name="ops")
            for kk in range(GD // 128):
                nc.tensor.matmul(
                    ps,
                    yT[:, kk, mt * 128 : (mt + 1) * 128],
                    wp_sb[:, kk, nh * 512 : (nh + 1) * 512],
                    start=(kk == 0),
                    stop=(kk == GD // 128 - 1),
                )
            nc.vector.tensor_copy(out=osb[:, nh * 512 : (nh + 1) * 512], in_=ps)
        nc.sync.dma_start(out=cc_in[mt * 128 : (mt + 1) * 128, :], in_=osb)


def _ar_chunk(nc, cc_in, cc_out, out, j, halves=1):
    """AllReduce + final output DMA for q-tile j's 512 token rows."""
    for v in range(halves):
        rows = slice(j * 512 + v * (512 // halves), j * 512 + (v + 1) * (512 // halves))
        nc.gpsimd.collective_compute(
            "AllReduce",
            mybir.AluOpType.add,
            replica_groups=[[0, 1], [2, 3], [4, 5], [6, 7]],
            ins=[cc_in[rows, :].opt()],
            outs=[cc_out[rows, :].opt()],
        )
    for mt in range(4 * j, 4 * j + 4):
        nc.sync.dma_start(
            out=out[mt * 128 : (mt + 1) * 128, :],
            in_=cc_out[mt * 128 : (mt + 1) * 128, :],
        )


def _build_nc():
    nc = bacc.Bacc(None, num_devices=8)

    xb = nc.dram_tensor("xb", [S, D], F32, kind="ExternalInput").ap()
    wqkv = nc.dram_tensor("wqkv", [D, 3 * GD], F32, kind="ExternalInput").ap()
    wproj = nc.dram_tensor("wproj", [GD, D], F32, kind="ExternalInput").ap()
    masks = nc.dram_tensor("masks", [128, 4, 1024], BF16, kind="ExternalInput").ap()
    ident = nc.dram_tensor("ident", [128, 128], F32, kind="ExternalInput").ap()
    out = nc.dram_tensor("out", [S, D], F32, kind="ExternalOutput").ap()

    with tile.TileContext(nc) as tc:
        with (
            tc.tile_pool(name="const", bufs=1) as const,
            tc.tile_pool(name="qk_p", bufs=1) as qk_p,
            tc.tile_pool(name="v_p", bufs=1) as v_p,
            tc.tile_pool(name="mm_ps", bufs=2, space="PSUM") as mm_ps,
            tc.tile_pool(name="dram", bufs=1, space="DRAM") as dram,
        ):
            identity = const.tile([128, 128], F32R)
            nc.sync.dma_start(out=identity, in_=ident.bitcast(F32R))
            mask_sb = const.tile([128, 4, 2, 512], BF16)
            nc.sync.dma_start(
                out=mask_sb, in_=masks.rearrange("p d (two q) -> p d two q", two=2)
            )

            # Q rows (m 0..3) and K rows (m 4..7), dim-major [512, 2048] each
            qkT = qk_p.tile([128, 2 * GD // 128, S], BF16)
            # V token-major with a ones column per head: [tok_tile, head, 65]
            v_sb = v_p.tile([128, NTOK, HL, 128], BF16)
            nc.vector.memset(v_sb[:, :, :, HD:128], 0.0)
            nc.vector.memset(v_sb[:, :, :, HD : HD + 1], 1.0)

            with (
                tc.tile_pool(name="wq_p", bufs=1) as wq_p,
                tc.tile_pool(name="xst_p", bufs=3) as xst_p,
                tc.tile_pool(name="xtn_p", bufs=2) as xtn_p,
                tc.tile_pool(name="tps_p", bufs=2, space="PSUM") as tps_p,
            ):
                _phase_qkv(
                    nc, wq_p, xst_p, xtn_p, tps_p, mm_ps,
                    wqkv, xb, identity, qkT, v_sb,
                )

            with (
                tc.tile_pool(name="yt_p", bufs=1) as yt_p,
                tc.tile_pool(name="probs_p", bufs=6) as probs_p,
                tc.tile_pool(name="den_p", bufs=4) as den_p,
                tc.tile_pool(name="wp_p", bufs=1) as wp_p,
                tc.tile_pool(name="out_p", bufs=3) as out_p,
                tc.tile_pool(name="y_ps", bufs=1, space="PSUM") as y_ps,
                tc.tile_pool(name="attn_ps", bufs=2, space="PSUM") as attn_ps,
            ):
                yT = yt_p.tile([128, GD // 128, S], F32R)
                wp_sb = wp_p.tile([128, GD // 128, D], F32R)
                for kk in range(GD // 128):
                    nc.sync.dma_start(
                        out=wp_sb[:, kk, :],
                        in_=wproj[kk * 128 : (kk + 1) * 128, :].bitcast(F32R),
                    )
                cc_in = dram.tile([S, D], F32)
                cc_out = dram.tile([S, D], F32)
                for j in range(NQT):
                    _attn_qtile(
                        nc, probs_p, den_p, attn_ps, y_ps,
                        mask_sb, qkT, v_sb, yT, j,
                    )
                    _proj_chunk(nc, out_p, mm_ps, wp_sb, yT, cc_in, j)
                    _ar_chunk(nc, cc_in, cc_out, out, j, halves=2 if j == NQT - 1 else 1)

    nc.compile()
    return nc


def _host_consts():
    ki = np.arange(128)[:, None, None]
    dd = np.arange(4)[None, :, None] * 128
    qj = np.arange(512)[None, None, :]
    m = (qj >= ki + dd).astype(np.float32)  # [128, 4, 512]
    masks = np.concatenate([m, m], axis=-1).astype(ml_dtypes.bfloat16)
    ident = np.eye(128, dtype=np.float32)
    return masks, ident


def _in_maps(x, w_qkv, w_proj):
    masks, ident = _host_consts()
    maps = []
    for c in range(8):
        b, g = c // 2, c % 2
        wq = w_qkv[:, g * GD : (g + 1) * GD]
        wk = w_qkv[:, D + g * GD : D + (g + 1) * GD]
        wv = w_qkv[:, 2 * D + g * GD : 2 * D + (g + 1) * GD]
        maps.append(
            {
                "xb": np.ascontiguousarray(x[b]),
                "wqkv": np.ascontiguousarray(np.concatenate([wq, wk, wv], axis=1)),
                "wproj": np.ascontiguousarray(w_proj[g * GD : (g + 1) * GD, :]),
                "masks": masks,
                "ident": ident,
            }
        )
    return maps


def kernel(x, w_qkv, w_proj):
    x = np.ascontiguousarray(x, dtype=np.float32)
    w_qkv = np.ascontiguousarray(w_qkv, dtype=np.float32)
    w_proj = np.ascontiguousarray(w_proj, dtype=np.float32)
    if "nc" not in _NC_CACHE:
        _NC_CACHE["nc"] = _build_nc()
    nc = _NC_CACHE["nc"]
    r = run_bass_kernel_spmd(nc, _in_maps(x, w_qkv, w_proj), list(range(8)))
    return np.stack([r.results[2 * b]["out"] for b in range(4)], axis=0)
